# revision 8
# baseline (speedup 1.0000x reference)
"""Trainium2 Bass kernel: Qwen3-style attention block (B=1, S=2048, D=2048,
H=16 q-heads, KV=8 kv-heads, HD=128), tensor-parallel over 8 NeuronCores.

Sharding: core c owns q-heads {2c, 2c+1} and kv-head c. Each core computes
its heads' Q/K/V projections, per-head RMSNorm + RoPE, masked softmax
attention (attn probs are a kernel output), attn@V, and a partial o_proj
(columns e = c*256..(c+1)*256 of the contraction). The host sums the 8
partial o_proj outputs (the all-reduce) and concatenates attn heads.

Device-side layouts (host pre-transposes so the contraction dim lands on
SBUF partitions everywhere):
  hT  (D, S)   hidden^T        qwT (D, 256)  q_w shard^T
  kwT (D,128)  vwT (D,128)     owT (256, D)  o_w shard^T
  wcq/wsq/wck/wsk (128, S)     folded RMS-weight x RoPE tables
"""

import functools

import numpy as np

import concourse.bacc as bacc
import concourse.mybir as mybir
import concourse.tile as tile
from concourse import bass_isa

S = 2048
D = 2048
HD = 128
H = 16
KV = 8
NCORES = 8
HPC = H // NCORES        # q heads per core = 2
EQ = HPC * HD            # 256 = per-core slice of the head dim
EPS = 1e-6

F32 = mybir.dt.float32
R32 = mybir.dt.float32r
AF = mybir.ActivationFunctionType
OP = mybir.AluOpType


def _r(ap):
    return ap.bitcast(R32)


def build_nc():
    nc = bacc.Bacc("TRN2", target_bir_lowering=False, debug=False)

    hT = nc.dram_tensor("hT", [D, S], F32, kind="ExternalInput")
    mask = nc.dram_tensor("mask", [S, S], F32, kind="ExternalInput")
    qwT = nc.dram_tensor("qwT", [D, EQ], F32, kind="ExternalInput")
    kwT = nc.dram_tensor("kwT", [D, HD], F32, kind="ExternalInput")
    vwT = nc.dram_tensor("vwT", [D, HD], F32, kind="ExternalInput")
    owT = nc.dram_tensor("owT", [EQ, D], F32, kind="ExternalInput")
    wtabs = {
        nm: nc.dram_tensor(nm, [HD, S], F32, kind="ExternalInput")
        for nm in ("wcq", "wsq", "wck", "wsk")
    }
    ident = nc.dram_tensor("ident", [128, 128], F32, kind="ExternalInput")

    attn_o = nc.dram_tensor("attn", [HPC, S, S], F32, kind="ExternalOutput")
    y_o = nc.dram_tensor("y", [S, D], F32, kind="ExternalOutput")

    with tile.TileContext(nc) as tc:
        with tc.tile_pool(name="persist", bufs=1) as pp:
            ident_s = pp.tile([128, 128], R32, tag="ident")
            nc.gpsimd.dma_start(out=ident_s[:], in_=ident[:])
            ident_f = ident_s[:].bitcast(F32)
            qrT = pp.tile([128, HPC * S], R32, tag="qrT")   # (hd, h*S+s)
            krT = pp.tile([128, S], R32, tag="krT")
            Vn = pp.tile([128, S], R32, tag="Vn")           # V natural, kt-major
            vTt = pp.tile([128, S], F32, tag="vTt")

            # ---------------- Phase 1: QKV projections ----------------
            with (
                tc.tile_pool(name="wpool", bufs=1) as wp,
                tc.tile_pool(name="pjps", bufs=4, space="PSUM") as pjp,
            ):
                hTs = wp.tile([128, 16 * S], R32, tag="hTs")
                qwTs = wp.tile([128, 16 * EQ], R32, tag="qwTs")
                kwTs = wp.tile([128, 16 * HD], R32, tag="kwTs")
                vwTs = wp.tile([128, 16 * HD], R32, tag="vwTs")
                for di in range(16):
                    dsl = slice(di * 128, (di + 1) * 128)
                    nc.gpsimd.dma_start(out=qwTs[:, di * EQ:(di + 1) * EQ], in_=qwT[dsl, :])
                    nc.gpsimd.dma_start(out=kwTs[:, di * HD:(di + 1) * HD], in_=kwT[dsl, :])
                    nc.gpsimd.dma_start(out=vwTs[:, di * HD:(di + 1) * HD], in_=vwT[dsl, :])
                # hidden^T, s-chunk-major so compute can start early
                for sc in range(4):
                    ssl = slice(sc * 512, (sc + 1) * 512)
                    for di in range(16):
                        nc.gpsimd.dma_start(
                            out=hTs[:, di * S + sc * 512: di * S + (sc + 1) * 512],
                            in_=hT[di * 128:(di + 1) * 128, ssl],
                        )
                # (dst column base fn, weight tile, weight row width, col offset)
                specs = [
                    (lambda sc: (qrT, 0 * S + sc * 512), qwTs, EQ, 0),
                    (lambda sc: (qrT, 1 * S + sc * 512), qwTs, EQ, 128),
                    (lambda sc: (krT, sc * 512), kwTs, HD, 0),
                    (lambda sc: (vTt, sc * 512), vwTs, HD, 0),
                ]
                for sc in range(4):
                    for dst_fn, wt, ww, wo in specs:
                        ps = pjp.tile([128, 512], F32)
                        for di in range(16):
                            nc.tensor.matmul(
                                ps[:],
                                wt[:, di * ww + wo: di * ww + wo + 128],
                                hTs[:, di * S + sc * 512: di * S + (sc + 1) * 512],
                                start=(di == 0),
                                stop=(di == 15),
                            )
                        dtile, c0 = dst_fn(sc)
                        nc.any.tensor_copy(dtile[:, c0:c0 + 512], ps[:])

            # ---------------- Phase 2: RMSNorm + RoPE + V transpose ----
            with (
                tc.tile_pool(name="tabs", bufs=1) as tbp,
                tc.tile_pool(name="rope", bufs=2) as rp2,
                tc.tile_pool(name="vps", bufs=2, space="PSUM") as vps,
            ):
                wts = {}
                for nm in ("wcq", "wsq", "wck", "wsk"):
                    t = tbp.tile([HD, S], F32, tag=nm, name=nm)
                    nc.sync.dma_start(out=t[:], in_=wtabs[nm][:])
                    wts[nm] = t
                bq = tbp.tile([128, 1], F32, tag="bq")
                nc.gpsimd.memset(bq[:], HD * EPS)
                bk = tbp.tile([128, 1], F32, tag="bk")
                nc.gpsimd.memset(bk[:], EPS)
                plans = [
                    (qrT, 0, wts["wcq"], wts["wsq"], True),
                    (qrT, S, wts["wcq"], wts["wsq"], True),
                    (krT, 0, wts["wck"], wts["wsk"], False),
                ]
                for xt, c0, wc, ws, isq in plans:
                    xa = xt[:, c0:c0 + S]
                    xa_f = xa.bitcast(F32)
                    sq = rp2.tile([128, S], F32, tag="sq")
                    nc.scalar.activation(sq[:], xa_f, AF.Square)
                    ms = rp2.tile([128, S], F32, tag="ms")
                    nc.gpsimd.partition_all_reduce(
                        ms[:], sq[:], 128, bass_isa.ReduceOp.add
                    )
                    # q: rb = 1/sqrt(ms + 128*eps) = scaling*rsqrt(var+eps)
                    # k: rb = 1/sqrt(ms/128 + eps) = rsqrt(var+eps)
                    sd = rp2.tile([128, S], F32, tag="sd")
                    nc.scalar.activation(
                        sd[:], ms[:], AF.Sqrt,
                        scale=(1.0 if isq else 1.0 / HD),
                        bias=(bq[:] if isq else bk[:]),
                    )
                    rb = rp2.tile([128, S], F32, tag="rb")
                    nc.vector.reciprocal(rb[:], sd[:])
                    xn = rp2.tile([128, S], F32, tag="xn")
                    nc.vector.tensor_tensor(out=xn[:], in0=xa_f, in1=rb[:], op=OP.mult)
                    t1 = rp2.tile([128, S], F32, tag="t1")
                    nc.vector.tensor_tensor(out=t1[:], in0=xn[:], in1=wc[:], op=OP.mult)
                    # rotate_half: partition shift by 64 via SBUF->SBUF DMA
                    xs = rp2.tile([128, S], F32, tag="xs")
                    nc.sync.dma_start(out=xs[0:64, :], in_=xn[64:128, :])
                    nc.sync.dma_start(out=xs[64:128, :], in_=xn[0:64, :])
                    nc.vector.tensor_tensor(out=xa, in0=xs[:], in1=ws[:], op=OP.mult)
                    nc.vector.tensor_tensor(
                        out=xa, in0=xa_f, in1=t1[:], op=OP.add)
                # V^T -> V natural (k on partitions)
                for g in range(4):
                    tp = vps.tile([128, 512], F32)
                    for j in range(4):
                        kt = 4 * g + j
                        nc.tensor.transpose(
                            tp[:, j * 128:(j + 1) * 128],
                            vTt[:, kt * 128:(kt + 1) * 128],
                            ident_f,
                        )
                    nc.any.tensor_copy(Vn[:, g * 512:(g + 1) * 512], tp[:])

            # ---------------- Phase 3: attention + o_proj ----------------
            with (
                tc.tile_pool(name="maskp", bufs=3) as mp,
                tc.tile_pool(name="attnp", bufs=4) as ap_,
                tc.tile_pool(name="stat", bufs=8) as stp,
                tc.tile_pool(name="ptp", bufs=1) as ptp,
                tc.tile_pool(name="owp", bufs=1) as owp,
                tc.tile_pool(name="yp", bufs=2) as yp_,
                tc.tile_pool(name="scps", bufs=1, space="PSUM") as scp,
                tc.tile_pool(name="tpps", bufs=2, space="PSUM") as tpp,
                tc.tile_pool(name="avps", bufs=1, space="PSUM") as avp,
                tc.tile_pool(name="ops", bufs=1, space="PSUM") as opp,
            ):
                outT = [owp.tile([128, S], R32, tag=f"outT{h}", name=f"outT{h}")
                        for h in range(HPC)]
                owTs = [owp.tile([HD, D], R32, tag=f"ow{e}", name=f"ow{e}")
                        for e in range(HPC)]
                for e in range(HPC):
                    nc.gpsimd.dma_start(out=owTs[e][:], in_=owT[e * 128:(e + 1) * 128, :])
                PT = [ptp.tile([128, 2 * S], R32, tag=f"pt{h}", name=f"pt{h}")
                      for h in range(HPC)]
                for qc in range(8):
                    mts = []
                    for off in range(2):
                        qt = 2 * qc + off
                        mt = mp.tile([128, S], R32, tag="mask")
                        nc.gpsimd.dma_start(out=mt[:], in_=mask[qt * 128:(qt + 1) * 128, :])
                        mts.append(mt)
                    for h in range(HPC):
                        for off in range(2):
                            qt = 2 * qc + off
                            ps = scp.tile([128, S], F32)
                            qsl = slice(h * S + qt * 128, h * S + qt * 128 + 128)
                            for kc in range(4):
                                sl = slice(kc * 512, (kc + 1) * 512)
                                nc.tensor.matmul(
                                    ps[:, sl], qrT[:, qsl], krT[:, sl],
                                    start=True, stop=False,
                                )
                                nc.tensor.matmul(
                                    ps[:, sl], ident_s[:], mts[off][:, sl],
                                    start=False, stop=True,
                                )
                            dt_ = stp.tile([128, 1], F32, tag="den")
                            nc.scalar.activation(ps[:], ps[:], AF.Exp, accum_out=dt_[:])
                            rt = stp.tile([128, 1], F32, tag="rcp")
                            nc.vector.reciprocal(rt[:], dt_[:])
                            at = ap_.tile([128, S], F32, tag="attn")
                            nc.scalar.activation(at[:], ps[:], AF.Copy, scale=rt[:])
                            nc.sync.dma_start(
                                out=attn_o[h, qt * 128:(qt + 1) * 128, :], in_=at[:]
                            )
                            for g in range(4):
                                tp = tpp.tile([128, 512], F32)
                                for j in range(4):
                                    kt = 4 * g + j
                                    nc.tensor.transpose(
                                        tp[:, j * 128:(j + 1) * 128],
                                        at[:, kt * 128:(kt + 1) * 128],
                                        ident_f,
                                    )
                                nc.any.tensor_copy(
                                    PT[h][:, off * S + g * 512: off * S + (g + 1) * 512],
                                    tp[:],
                                )
                        av = avp.tile([128, 256], F32)
                        pt3 = PT[h][:].rearrange("p (o s) -> p o s", o=2)
                        for kt in range(16):
                            nc.tensor.matmul(
                                av[:], Vn[:, kt * 128:(kt + 1) * 128],
                                pt3[:, :, kt * 128:(kt + 1) * 128],
                                start=(kt == 0), stop=(kt == 15),
                            )
                        nc.any.tensor_copy(outT[h][:, qc * 256:(qc + 1) * 256], av[:])
                    for off in range(2):
                        qt = 2 * qc + off
                        yt = yp_.tile([128, D], F32, tag="y")
                        for dmc in range(4):
                            yps = opp.tile([128, 512], F32)
                            for e in range(HPC):
                                nc.tensor.matmul(
                                    yps[:],
                                    outT[e][:, qt * 128:(qt + 1) * 128],
                                    owTs[e][:, dmc * 512:(dmc + 1) * 512],
                                    start=(e == 0), stop=(e == HPC - 1),
                                )
                            nc.any.tensor_copy(yt[:, dmc * 512:(dmc + 1) * 512], yps[:])
                        nc.sync.dma_start(out=y_o[qt * 128:(qt + 1) * 128, :], in_=yt[:])
    nc.compile()
    return nc


def host_prep(hidden_states, cos, sin, attention_mask, q_w, k_w, v_w, o_w,
              q_norm_w, k_norm_w):
    """Shard + layout inputs for the 8 cores. Pure numpy, cheap."""
    hid = np.ascontiguousarray(np.asarray(hidden_states, np.float32).reshape(S, D).T)
    mask2 = np.ascontiguousarray(
        np.broadcast_to(np.asarray(attention_mask, np.float32), (1, 1, S, S))[0, 0]
    )
    # fp32r rounding can overflow float32-min mask values to -inf, and the
    # identity-matmul mask add would then produce 0 * -inf = NaN. Any value
    # <= -104 already underflows exp() to 0 in fp32, so clamping is exact.
    mask2 = np.maximum(mask2, np.float32(-1e30))
    cosT = np.asarray(cos, np.float32).reshape(S, HD).T   # (HD, S)
    sinT = np.asarray(sin, np.float32).reshape(S, HD).T
    ident = np.eye(128, dtype=np.float32)

    def tabs(w):
        w = np.asarray(w, np.float32)
        wc = np.ascontiguousarray(w[:, None] * cosT)
        wsh = np.concatenate([w[64:], w[:64]])           # w[(p+64)%128]
        sgn = np.concatenate([-np.ones(64, np.float32), np.ones(64, np.float32)])
        ws = np.ascontiguousarray((sgn * wsh)[:, None] * sinT)
        return wc, ws

    wcq, wsq = tabs(q_norm_w)
    wck, wsk = tabs(k_norm_w)

    q_w = np.asarray(q_w, np.float32)
    k_w = np.asarray(k_w, np.float32)
    v_w = np.asarray(v_w, np.float32)
    o_w = np.asarray(o_w, np.float32)

    in_maps = []
    for c in range(NCORES):
        in_maps.append({
            "hT": hid,
            "mask": mask2,
            "qwT": np.ascontiguousarray(q_w[c * EQ:(c + 1) * EQ, :].T),
            "kwT": np.ascontiguousarray(k_w[c * HD:(c + 1) * HD, :].T),
            "vwT": np.ascontiguousarray(v_w[c * HD:(c + 1) * HD, :].T),
            "owT": np.ascontiguousarray(o_w[:, c * EQ:(c + 1) * EQ].T),
            "wcq": wcq, "wsq": wsq, "wck": wck, "wsk": wsk,
            "ident": ident,
        })
    return in_maps


class _Runner:
    """Compile once; run many times via PJRT shard_map over the 8 cores."""

    def __init__(self):
        import jax
        from jax.experimental.shard_map import shard_map
        from jax.sharding import Mesh, PartitionSpec
        from concourse import bass2jax

        self.jax = jax
        bass2jax.install_neuronx_cc_hook()
        nc = build_nc()
        self.nc = nc

        in_names, out_names, out_avals, zero_outs = [], [], [], []
        for alloc in nc.m.functions[0].allocations:
            if not isinstance(alloc, mybir.MemoryLocationSet):
                continue
            name = alloc.memorylocations[0].name
            pname = nc.partition_id_tensor.name if nc.partition_id_tensor else None
            if alloc.kind == "ExternalInput":
                if name != pname:
                    in_names.append(name)
            elif alloc.kind == "ExternalOutput":
                shape = tuple(alloc.tensor_shape)
                dtype = mybir.dt.np(alloc.dtype)
                out_names.append(name)
                out_avals.append(jax.core.ShapedArray(shape, dtype))
                zero_outs.append(np.zeros(shape, dtype))
        partition_name = (
            nc.partition_id_tensor.name if nc.partition_id_tensor else None
        )
        self.in_names = list(in_names)
        self.out_names = out_names
        self.zero_outs = zero_outs
        n_params = len(in_names)
        n_outs = len(out_names)
        all_in = in_names + out_names
        if partition_name is not None:
            all_in.append(partition_name)

        def _body(*args):
            operands = list(args)
            if partition_name is not None:
                operands.append(bass2jax.partition_id_tensor())
            outs = bass2jax._bass_exec_p.bind(
                *operands,
                out_avals=tuple(out_avals),
                in_names=tuple(all_in),
                out_names=tuple(out_names),
                lowering_input_output_aliases=(),
                sim_require_finite=True,
                sim_require_nnan=True,
                nc=nc,
            )
            return tuple(outs)

        devices = jax.devices()[:NCORES]
        mesh = Mesh(np.asarray(devices), ("core",))
        in_specs = (PartitionSpec("core"),) * (n_params + n_outs)
        out_specs = (PartitionSpec("core"),) * n_outs
        self.fn = jax.jit(
            shard_map(_body, mesh=mesh, in_specs=in_specs,
                      out_specs=out_specs, check_rep=False),
            donate_argnums=tuple(range(n_params, n_params + n_outs)),
            keep_unused=True,
        )

    def concat_inputs(self, in_maps):
        return [
            np.concatenate([np.asarray(in_maps[c][n]) for c in range(NCORES)], axis=0)
            for n in self.in_names
        ]

    def zero_buffers(self):
        return [
            np.zeros((NCORES * z.shape[0], *z.shape[1:]), z.dtype)
            for z in self.zero_outs
        ]

    def run(self, concat_in, zeros=None):
        if zeros is None:
            zeros = self.zero_buffers()
        out_arrs = self.fn(*concat_in, *zeros)
        return out_arrs


@functools.lru_cache(maxsize=1)
def _runner():
    return _Runner()


def kernel(**inputs):
    r = _runner()
    in_maps = host_prep(**inputs)
    concat_in = r.concat_inputs(in_maps)
    out_arrs = r.run(concat_in)
    named = dict(zip(r.out_names, out_arrs))
    attn_cat = np.asarray(named["attn"]).reshape(NCORES, HPC, S, S)
    y_cat = np.asarray(named["y"]).reshape(NCORES, S, D)
    attn = attn_cat.reshape(1, H, S, S)
    out = y_cat.sum(axis=0, dtype=np.float64).astype(np.float32).reshape(1, S, D)
    return out, attn
